# revision 36
# baseline (speedup 1.0000x reference)
"""Trainium2 Bass kernel for nn_HeatmapEncoder.

Math per (b, s, c) and per coordinate set (gaze, hand):
    g = exp(-((gx-cx)^2 + (gy-cy)^2) / (2 sigma^2))   on a 336x336 grid
    g = g / (sum(g) + eps)            (zeroed when cx+cy <= 0)
    unified = g_gaze + g_hand
    out = unified / (max(unified) + eps)

The Gaussian is separable, so each unified map is rank-2.  Each map is
generated ONCE by three K=6 bf16 matmuls (hi/lo split of each fp32
factor; the yl*xl term is dropped, rel err ~2^-16):
    rows (per set): (yh, xh), (yh, xl), (yl, xh)
Sum-normalization is folded into the y factors.

Peak normalization uses a COARSE pre-pass: a fourth small matmul per
map evaluates the map on a y-sub-3 x-sub-2 grid (112x168); its max
underestimates the true discrete peak by <= 1.6 % worst case; the
constant bias correction x1.0059 recenters the error to about +-1 %,
well inside the 2e-2 rel-err budget.  The reciprocal peak (DVE reduce
-> GPSIMD partition all-reduce -> DVE corr+eps -> DVE recip, batched
per 2 maps) is ready before the full map drains, so the drain is a
single fused scale+bf16-cast pass straight from PSUM (ACT takes chunks
0-1, DVE chunk 2), and the output is written to DRAM in bf16 (half the
DMA bytes; the host casts back to f32).

Layout: map j = 4*b + q keeps its 6 factor rows at SBUF partitions
32*q .. 32*q+5, free block b (PE row-tiles are tied to 32-aligned
partition groups; cycling q hides LDWEIGHTS under matmuls).  Map rows
are interleaved y = 3*p + c so each map is a single contiguous DRAM
range for the output DMA.  PSUM dests are 512-aligned (hardware
rejects matmul accumulation regions at unaligned bank offsets).

Sharding: pure data parallel over batch B=8 across the 8 cores.
"""

import functools
from contextlib import ExitStack

import numpy as np

try:
    import concourse.bass as bass
except ImportError:  # pragma: no cover
    import sys

    sys.path.insert(0, "/opt/trn_rl_repo")
    import concourse.bass as bass

import concourse.tile as tile
from concourse import bacc, bass_isa, mybir
from concourse.bass_utils import run_bass_kernel_spmd

H = W = 336
P = 112  # partitions per y-chunk; y = 3*p + c  (c in 0..2)
NCH = 3
S_DIM, C_DIM = 8, 4
NMAPS = S_DIM * C_DIM  # 32 maps per core
NR = 2 * NMAPS  # 64 factor rows (map-major, gaze/hand interleaved)
NB = 8  # free blocks in the aligned factor layout (map j = 4*b + q)
N_CORES = 8
SIGMA = 10.0 / 336.0
EXP_SCALE = -1.0 / (2.0 * SIGMA * SIGMA)
EPS = 1e-6
GROUP = 2
CW = 168  # coarse map x-resolution (x-sub-2); y-sub-3 via c=0 row slice
PKCORR = 1.0059  # recenters the coarse-peak underestimate (see docstring)

F32 = mybir.dt.float32
BF16 = mybir.dt.bfloat16
AF = mybir.ActivationFunctionType
ALU = mybir.AluOpType
AX = mybir.AxisListType


def _emit(nc, tc, ctx, negc_in, out_t, grid_const, ystg, xstg):
    const = ctx.enter_context(tc.tile_pool(name="const", bufs=1))
    fact = ctx.enter_context(tc.tile_pool(name="fact", bufs=1))
    ffac = ctx.enter_context(tc.tile_pool(name="ffac", bufs=1))
    small = ctx.enter_context(tc.tile_pool(name="small", bufs=2))
    sstage = ctx.enter_context(tc.tile_pool(name="sstage", bufs=4))
    pmap = ctx.enter_context(tc.tile_pool(name="pmap", bufs=2, space="PSUM"))
    cps = ctx.enter_context(tc.tile_pool(name="cps", bufs=2, space="PSUM"))

    # ---- early ACT table preload via dummy exp on a memset tile ----
    dum = small.tile([1, 16], F32, tag="dum")
    nc.gpsimd.memset(dum[:], 0.0)
    dum2 = small.tile([1, 16], F32, tag="dum2")
    nc.scalar.activation(dum2[:], dum[:], AF.Exp, bias=0.0, scale=1.0)

    # ---- constants / inputs: the tiny negc DMA has multi-us latency,
    # so it issues first (on the scalar queue, ahead of the table load) ----
    NC2 = const.tile([NR, 2], F32)
    nc.scalar.dma_start(NC2[:], negc_in.ap())
    G = const.tile([NR, W], F32)
    nc.sync.dma_start(G[:], grid_const.ap())

    # ---- 1-D gaussian factors, dense [64, 336] fp32 (x side first:
    # the x factors gate the scatters) ----
    sqx = fact.tile([NR, W], F32)
    nc.scalar.activation(sqx[:], G[:], AF.Square, bias=NC2[:, 0:1], scale=1.0)
    fxv = fact.tile([NR, W], F32)
    nc.scalar.activation(fxv[:], sqx[:], AF.Exp, bias=0.0, scale=EXP_SCALE)
    sqy = fact.tile([NR, W], F32)
    nc.scalar.activation(sqy[:], G[:], AF.Square, bias=NC2[:, 1:2], scale=1.0)
    fyv = fact.tile([NR, W], F32)
    nc.scalar.activation(fyv[:], sqy[:], AF.Exp, bias=0.0, scale=EXP_SCALE)

    # x-side hi/lo split (UNSCALED - off the normalization chain, so the
    # x scatters can start early); the a-scale folds into the y side below
    xh = fact.tile([NR, W], BF16)
    nc.vector.tensor_copy(xh[:], fxv[:])
    xl = fact.tile([NR, W], BF16)
    nc.vector.tensor_sub(xl[:], fxv[:], xh[:])

    # normalization scale a = valid / (Sx*Sy + eps) folded into y factors
    sx = small.tile([NR, 1], F32, tag="sx")
    nc.vector.reduce_sum(sx[:], fxv[:], axis=AX.X)
    sy = small.tile([NR, 1], F32, tag="sy")
    nc.vector.reduce_sum(sy[:], fyv[:], axis=AX.X)
    ss = small.tile([NR, 1], F32, tag="ss")
    nc.vector.tensor_mul(ss[:], sx[:], sy[:])
    rec = small.tile([NR, 1], F32, tag="rec")
    nc.vector.reciprocal(rec[:], ss[:])  # ref's +eps is a 1.7e-9 effect
    vs = small.tile([NR, 1], F32, tag="vs")
    nc.vector.tensor_add(vs[:], NC2[:, 0:1], NC2[:, 1:2])
    vm = small.tile([NR, 1], F32, tag="vm")  # valid: (-cx)+(-cy) < 0
    nc.vector.tensor_scalar(vm[:], vs[:], 0.0, None, op0=ALU.is_lt)
    av = small.tile([NR, 1], F32, tag="av")
    nc.vector.tensor_mul(av[:], rec[:], vm[:])

    # y-side hi/lo split (carries the a-scale), fused: no fys round-trip
    yh = fact.tile([NR, W], BF16)
    nc.vector.tensor_scalar_mul(yh[:], fyv[:], av[:, 0:1])
    yl = fact.tile([NR, W], BF16)
    nc.vector.scalar_tensor_tensor(yl[:], fyv[:], av[:, 0:1], yh[:],
                                   op0=ALU.mult, op1=ALU.subtract)

    # ---- bounce through DRAM into the 32-aligned 6-row layout ----
    # staging [3, 64, 336]: x first (ready early); y side (yh, yh, yl)
    nc.sync.dma_start(xstg.ap()[0], xh[:])
    nc.scalar.dma_start(xstg.ap()[1], xl[:])
    nc.scalar.dma_start(xstg.ap()[2], xh[:])
    nc.sync.dma_start(ystg.ap()[0], yh[:])
    nc.scalar.dma_start(ystg.ap()[1], yh[:])
    nc.sync.dma_start(ystg.ap()[2], yl[:])

    # gather DMAs: dst rows 32q+3t+u <- stg[u, 8b+2q+t, x]
    FY = [ffac.tile([128, NB, W], BF16, name=f"FY{q}", tag=f"fy{q}")
          for q in range(4)]
    FX = [ffac.tile([128, NB, W], BF16, name=f"FX{q}", tag=f"fx{q}")
          for q in range(4)]
    ya = ystg.ap()  # [3, 64, 336]
    xa = xstg.ap()
    qeng = (nc.sync, nc.scalar, nc.sync, nc.scalar)
    for q in range(4):
        for t in range(2):
            r0 = 32 * q + 3 * t
            qeng[q].dma_start(FY[q][r0:r0 + 3, :, :], ya[:, 2 * q + t::8, :])
            qeng[(q + 1) % 4].dma_start(FX[q][r0:r0 + 3, :, :],
                                        xa[:, 2 * q + t::8, :])

    # output stays in the stage layout [p, m, (c x)] (y = 3p+c): each
    # partition line is one contiguous DRAM run per DMA, so descriptor
    # dispatch (the output-queue bottleneck) halves; host untangles it
    dview = out_t.ap()

    pts = {}
    for j0 in range(0, NMAPS, GROUP):
        mb = small.tile([P, GROUP], F32, tag="mb")
        # coarse matmuls for the whole group first (maximal rg lookahead)
        cts = {}
        for j in range(j0, j0 + GROUP):
            q, b = j % 4, j // 4
            ct = cps.tile([P, 512], F32, tag="ct")
            nc.tensor.matmul(ct[:, 0:CW], FY[q][32 * q:32 * q + 6, b, 0::3],
                             FX[q][32 * q:32 * q + 6, b, 0::2],
                             start=True, stop=True,
                             tile_position=(32 * q, 0))
            cts[j] = ct
        for j in range(j0, j0 + GROUP):
            q, b = j % 4, j // 4
            pt = pmap.tile([P, NCH * 512], F32, tag="pmap")
            rhs = FX[q][32 * q:32 * q + 6, b, :]
            for cix in range(NCH):
                lhsT = FY[q][32 * q:32 * q + 6, b, cix::3]
                nc.tensor.matmul(pt[:, cix * 512:cix * 512 + W], lhsT, rhs,
                                 start=True, stop=True,
                                 tile_position=(32 * q, 0))
            pts[j] = pt
            nc.vector.reduce_max(mb[:, j - j0:j - j0 + 1], cts[j][:, 0:CW],
                                 axis=AX.X)

        # peak chain: rg = 1/(allreduce_max(mb)*PKCORR + EPS)
        par = small.tile([P, GROUP], F32, tag="par")
        nc.gpsimd.partition_all_reduce(par[:], mb[:], channels=P,
                                       reduce_op=bass_isa.ReduceOp.max)
        pke = small.tile([P, GROUP], F32, tag="pke")
        nc.vector.tensor_scalar(pke[:], par[:], PKCORR, EPS,
                                op0=ALU.mult, op1=ALU.add)
        rg = small.tile([P, GROUP], F32, tag="rg")
        nc.vector.reciprocal(rg[:], pke[:])

        # fused scale+cast drain straight out of PSUM: ACT chunks 0-1,
        # DVE chunk 2; then one output DMA per group
        st = sstage.tile([P, GROUP, NCH * W], BF16, tag="sst")
        for j in range(j0, j0 + GROUP):
            gi = j - j0
            pview = pts[j][:].rearrange("p (c z) -> p c z", c=NCH)[:, :, 0:W]
            sview = st[:, gi, :].rearrange("p (c x) -> p c x", c=NCH)
            nc.vector.tensor_scalar_mul(sview[:, 2, :], pview[:, 2, :],
                                        rg[:, gi:gi + 1])
            nc.scalar.mul(sview[:, 0:2, :], pview[:, 0:2, :],
                          rg[:, gi:gi + 1])
            del pts[j]
        nc.sync.dma_start(dview[:, j0:j0 + GROUP, :], st[:])


@functools.lru_cache(maxsize=1)
def _build():
    nc = bacc.Bacc("TRN2", target_bir_lowering=False, debug=False)
    negc_in = nc.dram_tensor("negc", [NR, 2], F32, kind="ExternalInput")
    out_t = nc.dram_tensor("out", [P, NMAPS, NCH * W], BF16,
                           kind="ExternalOutput")

    grid = (np.arange(W, dtype=np.float64) / (W - 1)).astype(np.float32)
    grid_const = nc.inline_tensor(np.tile(grid, (NR, 1)), name="gridc")

    ystg = nc.dram_tensor("ystg", [3, NR, W], BF16)
    xstg = nc.dram_tensor("xstg", [3, NR, W], BF16)

    with tile.TileContext(nc) as tc, ExitStack() as ctx:
        _emit(nc, tc, ctx, negc_in, out_t, grid_const, ystg, xstg)
    nc.compile()
    return nc


def _in_map_for(gaze, hand, b):
    cg = np.asarray(gaze[b], dtype=np.float32).reshape(NMAPS, 2)
    ch = np.asarray(hand[b], dtype=np.float32).reshape(NMAPS, 2)
    inter = np.stack([cg, ch], axis=1).reshape(NR, 2)  # row 2*j + t
    return {"negc": np.ascontiguousarray(-inter)}


def kernel(gaze_coords, hand_coords, _trace=False, **trace_kwargs):
    gaze_coords = np.asarray(gaze_coords, dtype=np.float32)
    hand_coords = np.asarray(hand_coords, dtype=np.float32)
    B = gaze_coords.shape[0]
    assert B == N_CORES, f"expected batch {N_CORES}, got {B}"
    nc = _build()
    in_maps = [_in_map_for(gaze_coords, hand_coords, b) for b in range(B)]
    res = run_bass_kernel_spmd(nc, in_maps, list(range(N_CORES)),
                               trace=_trace, **trace_kwargs)
    def _untangle(o):
        # [P, NMAPS, NCH*W] -> [NMAPS, H, W] with y = 3p+c
        a = np.asarray(o, dtype=np.float32).reshape(P, NMAPS, NCH, W)
        return a.transpose(1, 0, 2, 3).reshape(NMAPS, H, W)

    out = np.stack(
        [_untangle(res.results[i]["out"]).reshape(S_DIM, C_DIM, H, W)
         for i in range(B)],
        axis=0,
    )
    if _trace:
        return out, res
    return out


# revision 38
# speedup vs baseline: 1.0102x; 1.0102x over previous
"""Trainium2 Bass kernel for nn_HeatmapEncoder.

Math per (b, s, c) and per coordinate set (gaze, hand):
    g = exp(-((gx-cx)^2 + (gy-cy)^2) / (2 sigma^2))   on a 336x336 grid
    g = g / (sum(g) + eps)            (zeroed when cx+cy <= 0)
    unified = g_gaze + g_hand
    out = unified / (max(unified) + eps)

The Gaussian is separable, so each unified map is rank-2.  Each map is
generated ONCE by three K=6 bf16 matmuls (hi/lo split of each fp32
factor; the yl*xl term is dropped, rel err ~2^-16):
    rows (per set): (yh, xh), (yh, xl), (yl, xh)
Sum-normalization is folded into the y factors.

Peak normalization uses a COARSE pre-pass: a fourth small matmul per
map evaluates the map on a y-sub-3 x-sub-2 grid (112x168); its max
underestimates the true discrete peak by <= 1.6 % worst case; the
constant bias correction x1.0059 recenters the error to about +-1 %,
well inside the 2e-2 rel-err budget.  The reciprocal peak (DVE reduce
-> GPSIMD partition all-reduce -> DVE corr+eps -> DVE recip, batched
per 2 maps) is ready before the full map drains, so the drain is a
single fused scale+bf16-cast pass straight from PSUM (ACT takes chunks
0-1, DVE chunk 2), and the output is written to DRAM in bf16 (half the
DMA bytes; the host casts back to f32).

Layout: map j = 4*b + q keeps its 6 factor rows at SBUF partitions
32*q .. 32*q+5, free block b (PE row-tiles are tied to 32-aligned
partition groups; cycling q hides LDWEIGHTS under matmuls).  Map rows
are interleaved y = 3*p + c so each map is a single contiguous DRAM
range for the output DMA.  PSUM dests are 512-aligned (hardware
rejects matmul accumulation regions at unaligned bank offsets).

Sharding: pure data parallel over batch B=8 across the 8 cores.
"""

import functools
from contextlib import ExitStack

import numpy as np

try:
    import concourse.bass as bass
except ImportError:  # pragma: no cover
    import sys

    sys.path.insert(0, "/opt/trn_rl_repo")
    import concourse.bass as bass

import concourse.tile as tile
from concourse import bacc, bass_isa, mybir
from concourse.bass_utils import run_bass_kernel_spmd

H = W = 336
P = 112  # partitions per y-chunk; y = 3*p + c  (c in 0..2)
NCH = 3
S_DIM, C_DIM = 8, 4
NMAPS = S_DIM * C_DIM  # 32 maps per core
NR = 2 * NMAPS  # 64 factor rows (map-major, gaze/hand interleaved)
NB = 8  # free blocks in the aligned factor layout (map j = 4*b + q)
N_CORES = 8
SIGMA = 10.0 / 336.0
EXP_SCALE = -1.0 / (2.0 * SIGMA * SIGMA)
EPS = 1e-6
GROUP = 2
CW = 168  # coarse map x-resolution (x-sub-2); y-sub-3 via c=0 row slice
PKCORR = 1.0059  # recenters the coarse-peak underestimate (see docstring)

F32 = mybir.dt.float32
BF16 = mybir.dt.bfloat16
AF = mybir.ActivationFunctionType
ALU = mybir.AluOpType
AX = mybir.AxisListType


def _emit(nc, tc, ctx, negc_in, out_t, grid_const, ystg, xstg):
    const = ctx.enter_context(tc.tile_pool(name="const", bufs=1))
    fact = ctx.enter_context(tc.tile_pool(name="fact", bufs=1))
    ffac = ctx.enter_context(tc.tile_pool(name="ffac", bufs=1))
    small = ctx.enter_context(tc.tile_pool(name="small", bufs=2))
    sstage = ctx.enter_context(tc.tile_pool(name="sstage", bufs=4))
    pmap = ctx.enter_context(tc.tile_pool(name="pmap", bufs=2, space="PSUM"))
    cps = ctx.enter_context(tc.tile_pool(name="cps", bufs=2, space="PSUM"))

    # ---- early ACT table preload via dummy exp on a memset tile ----
    dum = small.tile([1, 16], F32, tag="dum")
    nc.gpsimd.memset(dum[:], 0.0)
    dum2 = small.tile([1, 16], F32, tag="dum2")
    nc.scalar.activation(dum2[:], dum[:], AF.Exp, bias=0.0, scale=1.0)
    epst = const.tile([P, 1], F32)
    nc.gpsimd.memset(epst[:], EPS)

    # ---- constants / inputs: the tiny negc DMA has multi-us latency,
    # so it issues first (on the scalar queue, ahead of the table load) ----
    NC2 = const.tile([NR, 2], F32)
    nc.scalar.dma_start(NC2[:], negc_in.ap())
    G = const.tile([NR, W], F32)
    nc.sync.dma_start(G[:], grid_const.ap())

    # ---- 1-D gaussian factors, dense [64, 336] fp32 (x side first:
    # the x factors gate the scatters) ----
    sqx = fact.tile([NR, W], F32)
    nc.scalar.activation(sqx[:], G[:], AF.Square, bias=NC2[:, 0:1], scale=1.0)
    fxv = fact.tile([NR, W], F32)
    nc.scalar.activation(fxv[:], sqx[:], AF.Exp, bias=0.0, scale=EXP_SCALE)
    sqy = fact.tile([NR, W], F32)
    nc.scalar.activation(sqy[:], G[:], AF.Square, bias=NC2[:, 1:2], scale=1.0)
    fyv = fact.tile([NR, W], F32)
    nc.scalar.activation(fyv[:], sqy[:], AF.Exp, bias=0.0, scale=EXP_SCALE)

    # x-side hi/lo split (UNSCALED - off the normalization chain, so the
    # x scatters can start early); the a-scale folds into the y side below
    xh = fact.tile([NR, W], BF16)
    nc.vector.tensor_copy(xh[:], fxv[:])
    xl = fact.tile([NR, W], BF16)
    nc.vector.tensor_sub(xl[:], fxv[:], xh[:])

    # normalization scale a = valid / (Sx*Sy + eps) folded into y factors
    sx = small.tile([NR, 1], F32, tag="sx")
    nc.vector.reduce_sum(sx[:], fxv[:], axis=AX.X)
    sy = small.tile([NR, 1], F32, tag="sy")
    nc.vector.reduce_sum(sy[:], fyv[:], axis=AX.X)
    ss = small.tile([NR, 1], F32, tag="ss")
    nc.vector.tensor_mul(ss[:], sx[:], sy[:])
    rec = small.tile([NR, 1], F32, tag="rec")
    nc.vector.reciprocal(rec[:], ss[:])  # ref's +eps is a 1.7e-9 effect
    vs = small.tile([NR, 1], F32, tag="vs")
    nc.vector.tensor_add(vs[:], NC2[:, 0:1], NC2[:, 1:2])
    vm = small.tile([NR, 1], F32, tag="vm")  # valid: (-cx)+(-cy) < 0
    nc.vector.tensor_scalar(vm[:], vs[:], 0.0, None, op0=ALU.is_lt)
    av = small.tile([NR, 1], F32, tag="av")
    nc.vector.tensor_mul(av[:], rec[:], vm[:])

    # y-side hi/lo split (carries the a-scale), fused: no fys round-trip
    yh = fact.tile([NR, W], BF16)
    nc.vector.tensor_scalar_mul(yh[:], fyv[:], av[:, 0:1])
    yl = fact.tile([NR, W], BF16)
    nc.vector.scalar_tensor_tensor(yl[:], fyv[:], av[:, 0:1], yh[:],
                                   op0=ALU.mult, op1=ALU.subtract)

    # ---- bounce through DRAM into the 32-aligned 6-row layout ----
    # staging [3, 64, 336]: x first (ready early); y side (yh, yh, yl)
    nc.sync.dma_start(xstg.ap()[0], xh[:])
    nc.scalar.dma_start(xstg.ap()[1], xl[:])
    nc.scalar.dma_start(xstg.ap()[2], xh[:])
    nc.sync.dma_start(ystg.ap()[0], yh[:])
    nc.scalar.dma_start(ystg.ap()[1], yh[:])
    nc.sync.dma_start(ystg.ap()[2], yl[:])

    # gather DMAs: dst rows 32q+3t+u <- stg[u, 8b+2q+t, x]
    FY = [ffac.tile([128, NB, W], BF16, name=f"FY{q}", tag=f"fy{q}")
          for q in range(4)]
    FX = [ffac.tile([128, NB, W], BF16, name=f"FX{q}", tag=f"fx{q}")
          for q in range(4)]
    ya = ystg.ap()  # [3, 64, 336]
    xa = xstg.ap()
    qeng = (nc.sync, nc.scalar, nc.sync, nc.scalar)
    for q in range(4):
        for t in range(2):
            r0 = 32 * q + 3 * t
            qeng[q].dma_start(FY[q][r0:r0 + 3, :, :], ya[:, 2 * q + t::8, :])
            qeng[(q + 1) % 4].dma_start(FX[q][r0:r0 + 3, :, :],
                                        xa[:, 2 * q + t::8, :])

    # output stays in the stage layout [p, m, (c x)] (y = 3p+c): each
    # partition line is one contiguous DRAM run per DMA, so descriptor
    # dispatch (the output-queue bottleneck) halves; host untangles it
    dview = out_t.ap()

    pts = {}
    for j0 in range(0, NMAPS, GROUP):
        mb = small.tile([P, GROUP], F32, tag="mb")
        # coarse matmuls for the whole group first (maximal rg lookahead)
        cts = {}
        for j in range(j0, j0 + GROUP):
            q, b = j % 4, j // 4
            ct = cps.tile([P, 512], F32, tag="ct")
            nc.tensor.matmul(ct[:, 0:CW], FY[q][32 * q:32 * q + 6, b, 0::3],
                             FX[q][32 * q:32 * q + 6, b, 0::2],
                             start=True, stop=True,
                             tile_position=(32 * q, 0))
            cts[j] = ct
        for j in range(j0, j0 + GROUP):
            q, b = j % 4, j // 4
            pt = pmap.tile([P, NCH * 512], F32, tag="pmap")
            rhs = FX[q][32 * q:32 * q + 6, b, :]
            for cix in range(NCH):
                lhsT = FY[q][32 * q:32 * q + 6, b, cix::3]
                nc.tensor.matmul(pt[:, cix * 512:cix * 512 + W], lhsT, rhs,
                                 start=True, stop=True,
                                 tile_position=(32 * q, 0))
            pts[j] = pt
            nc.vector.reduce_max(mb[:, j - j0:j - j0 + 1], cts[j][:, 0:CW],
                                 axis=AX.X)

        # peak chain: rg = 1/(allreduce_max(mb)*PKCORR + EPS)
        par = small.tile([P, GROUP], F32, tag="par")
        nc.gpsimd.partition_all_reduce(par[:], mb[:], channels=P,
                                       reduce_op=bass_isa.ReduceOp.max)
        pke = small.tile([P, GROUP], F32, tag="pke")
        # on ACT (Identity(par*PKCORR + EPS)): keeps the gpsimd wait off
        # the DVE queue, which paces the PSUM-recycle loop
        nc.scalar.activation(pke[:], par[:], AF.Identity, bias=epst[:, 0:1],
                             scale=PKCORR)
        rg = small.tile([P, GROUP], F32, tag="rg")
        nc.vector.reciprocal(rg[:], pke[:])

        # fused scale+cast drain straight out of PSUM: ACT chunks 0-1,
        # DVE chunk 2; then one output DMA per group
        st = sstage.tile([P, GROUP, NCH * W], BF16, tag="sst")
        for j in range(j0, j0 + GROUP):
            gi = j - j0
            pview = pts[j][:].rearrange("p (c z) -> p c z", c=NCH)[:, :, 0:W]
            sview = st[:, gi, :].rearrange("p (c x) -> p c x", c=NCH)
            nc.vector.tensor_scalar_mul(sview[:, 2, :], pview[:, 2, :],
                                        rg[:, gi:gi + 1])
            nc.scalar.mul(sview[:, 0:2, :], pview[:, 0:2, :],
                          rg[:, gi:gi + 1])
            del pts[j]
        nc.sync.dma_start(dview[:, j0:j0 + GROUP, :], st[:])


@functools.lru_cache(maxsize=1)
def _build():
    nc = bacc.Bacc("TRN2", target_bir_lowering=False, debug=False)
    negc_in = nc.dram_tensor("negc", [NR, 2], F32, kind="ExternalInput")
    out_t = nc.dram_tensor("out", [P, NMAPS, NCH * W], BF16,
                           kind="ExternalOutput")

    grid = (np.arange(W, dtype=np.float64) / (W - 1)).astype(np.float32)
    grid_const = nc.inline_tensor(np.tile(grid, (NR, 1)), name="gridc")

    ystg = nc.dram_tensor("ystg", [3, NR, W], BF16)
    xstg = nc.dram_tensor("xstg", [3, NR, W], BF16)

    with tile.TileContext(nc) as tc, ExitStack() as ctx:
        _emit(nc, tc, ctx, negc_in, out_t, grid_const, ystg, xstg)
    nc.compile()
    return nc


def _in_map_for(gaze, hand, b):
    cg = np.asarray(gaze[b], dtype=np.float32).reshape(NMAPS, 2)
    ch = np.asarray(hand[b], dtype=np.float32).reshape(NMAPS, 2)
    inter = np.stack([cg, ch], axis=1).reshape(NR, 2)  # row 2*j + t
    return {"negc": np.ascontiguousarray(-inter)}


def kernel(gaze_coords, hand_coords, _trace=False, **trace_kwargs):
    gaze_coords = np.asarray(gaze_coords, dtype=np.float32)
    hand_coords = np.asarray(hand_coords, dtype=np.float32)
    B = gaze_coords.shape[0]
    assert B == N_CORES, f"expected batch {N_CORES}, got {B}"
    nc = _build()
    in_maps = [_in_map_for(gaze_coords, hand_coords, b) for b in range(B)]
    res = run_bass_kernel_spmd(nc, in_maps, list(range(N_CORES)),
                               trace=_trace, **trace_kwargs)
    def _untangle(o):
        # [P, NMAPS, NCH*W] -> [NMAPS, H, W] with y = 3p+c
        a = np.asarray(o, dtype=np.float32).reshape(P, NMAPS, NCH, W)
        return a.transpose(1, 0, 2, 3).reshape(NMAPS, H, W)

    out = np.stack(
        [_untangle(res.results[i]["out"]).reshape(S_DIM, C_DIM, H, W)
         for i in range(B)],
        axis=0,
    )
    if _trace:
        return out, res
    return out


# revision 41
# speedup vs baseline: 1.0192x; 1.0089x over previous
"""Trainium2 Bass kernel for nn_HeatmapEncoder.

Math per (b, s, c) and per coordinate set (gaze, hand):
    g = exp(-((gx-cx)^2 + (gy-cy)^2) / (2 sigma^2))   on a 336x336 grid
    g = g / (sum(g) + eps)            (zeroed when cx+cy <= 0)
    unified = g_gaze + g_hand
    out = unified / (max(unified) + eps)

The Gaussian is separable, so each unified map is rank-2.  Each map is
generated ONCE by three K=6 bf16 matmuls (hi/lo split of each fp32
factor; the yl*xl term is dropped, rel err ~2^-16):
    rows (per set): (yh, xh), (yh, xl), (yl, xh)
Sum-normalization is folded into the y factors.

Peak normalization uses a COARSE pre-pass: a fourth small matmul per
map evaluates the map on a y-sub-3 x-sub-2 grid (112x168); its max
underestimates the true discrete peak by <= 1.6 % worst case; the
constant bias correction x1.0059 recenters the error to about +-1 %,
well inside the 2e-2 rel-err budget.  The reciprocal peak (DVE reduce
-> GPSIMD partition all-reduce -> DVE corr+eps -> DVE recip, batched
per 2 maps) is ready before the full map drains, so the drain is a
single fused scale+bf16-cast pass straight from PSUM (ACT takes chunks
0-1, DVE chunk 2), and the output is written to DRAM in bf16 (half the
DMA bytes; the host casts back to f32).

Layout: map j = 4*b + q keeps its 6 factor rows at SBUF partitions
32*q .. 32*q+5, free block b (PE row-tiles are tied to 32-aligned
partition groups; cycling q hides LDWEIGHTS under matmuls).  Map rows
are interleaved y = 3*p + c so each map is a single contiguous DRAM
range for the output DMA.  PSUM dests are 512-aligned (hardware
rejects matmul accumulation regions at unaligned bank offsets).

Sharding: pure data parallel over batch B=8 across the 8 cores.
"""

import functools
from contextlib import ExitStack

import numpy as np

try:
    import concourse.bass as bass
except ImportError:  # pragma: no cover
    import sys

    sys.path.insert(0, "/opt/trn_rl_repo")
    import concourse.bass as bass

import concourse.tile as tile
from concourse import bacc, bass_isa, mybir
from concourse.bass_utils import run_bass_kernel_spmd

H = W = 336
P = 112  # partitions per y-chunk; y = 3*p + c  (c in 0..2)
NCH = 3
S_DIM, C_DIM = 8, 4
NMAPS = S_DIM * C_DIM  # 32 maps per core
NR = 2 * NMAPS  # 64 factor rows (map-major, gaze/hand interleaved)
NB = 8  # free blocks in the aligned factor layout (map j = 4*b + q)
N_CORES = 8
SIGMA = 10.0 / 336.0
EXP_SCALE = -1.0 / (2.0 * SIGMA * SIGMA)
EPS = 1e-6
GROUP = 2
CW = 168  # coarse map x-resolution (x-sub-2); y-sub-3 via c=0 row slice
PKCORR = 1.0059  # recenters the coarse-peak underestimate (see docstring)

F32 = mybir.dt.float32
BF16 = mybir.dt.bfloat16
AF = mybir.ActivationFunctionType
ALU = mybir.AluOpType
AX = mybir.AxisListType


def _emit(nc, tc, ctx, negc_in, out_t, grid_const, ystg, xstg):
    const = ctx.enter_context(tc.tile_pool(name="const", bufs=1))
    fact = ctx.enter_context(tc.tile_pool(name="fact", bufs=1))
    ffac = ctx.enter_context(tc.tile_pool(name="ffac", bufs=1))
    small = ctx.enter_context(tc.tile_pool(name="small", bufs=2))
    sstage = ctx.enter_context(tc.tile_pool(name="sstage", bufs=4))
    pmap = ctx.enter_context(tc.tile_pool(name="pmap", bufs=2, space="PSUM"))
    cps = ctx.enter_context(tc.tile_pool(name="cps", bufs=2, space="PSUM"))

    # ---- early ACT table preload via dummy exp on a memset tile ----
    dum = small.tile([1, 16], F32, tag="dum")
    nc.gpsimd.memset(dum[:], 0.0)
    dum2 = small.tile([1, 16], F32, tag="dum2")
    nc.scalar.activation(dum2[:], dum[:], AF.Exp, bias=0.0, scale=1.0)
    epst = const.tile([P, 1], F32)
    nc.gpsimd.memset(epst[:], EPS)

    # ---- constants / inputs: the tiny negc DMA has multi-us latency,
    # so it issues first (on the scalar queue, ahead of the table load) ----
    NC2 = const.tile([NR, 2], F32)
    nc.scalar.dma_start(NC2[:], negc_in.ap())
    G = const.tile([NR, W], F32)
    nc.sync.dma_start(G[:], grid_const.ap())

    # ---- 1-D gaussian factors, dense [64, 336] fp32 (x side first:
    # the x factors gate the scatters) ----
    sqx = fact.tile([NR, W], F32)
    nc.scalar.activation(sqx[:], G[:], AF.Square, bias=NC2[:, 0:1], scale=1.0)
    fxv = fact.tile([NR, W], F32)
    nc.scalar.activation(fxv[:], sqx[:], AF.Exp, bias=0.0, scale=EXP_SCALE)
    sqy = fact.tile([NR, W], F32)
    nc.scalar.activation(sqy[:], G[:], AF.Square, bias=NC2[:, 1:2], scale=1.0)
    fyv = fact.tile([NR, W], F32)
    nc.scalar.activation(fyv[:], sqy[:], AF.Exp, bias=0.0, scale=EXP_SCALE)

    # x-side hi/lo split (UNSCALED - off the normalization chain, so the
    # x scatters can start early); the a-scale folds into the y side below
    xh = fact.tile([NR, W], BF16)
    nc.vector.tensor_copy(xh[:], fxv[:])
    xl = fact.tile([NR, W], BF16)
    nc.vector.tensor_sub(xl[:], fxv[:], xh[:])

    # normalization scale a = valid / (Sx*Sy + eps) folded into y factors
    sx = small.tile([NR, 1], F32, tag="sx")
    nc.vector.reduce_sum(sx[:], fxv[:], axis=AX.X)
    sy = small.tile([NR, 1], F32, tag="sy")
    nc.vector.reduce_sum(sy[:], fyv[:], axis=AX.X)
    ss = small.tile([NR, 1], F32, tag="ss")
    nc.vector.tensor_mul(ss[:], sx[:], sy[:])
    rec = small.tile([NR, 1], F32, tag="rec")
    nc.vector.reciprocal(rec[:], ss[:])  # ref's +eps is a 1.7e-9 effect
    vs = small.tile([NR, 1], F32, tag="vs")
    nc.vector.tensor_add(vs[:], NC2[:, 0:1], NC2[:, 1:2])
    vm = small.tile([NR, 1], F32, tag="vm")  # valid: (-cx)+(-cy) < 0
    nc.vector.tensor_scalar(vm[:], vs[:], 0.0, None, op0=ALU.is_lt)
    av = small.tile([NR, 1], F32, tag="av")
    nc.vector.tensor_mul(av[:], rec[:], vm[:])

    # y-side hi/lo split (carries the a-scale), fused: no fys round-trip
    yh = fact.tile([NR, W], BF16)
    nc.vector.tensor_scalar_mul(yh[:], fyv[:], av[:, 0:1])
    yl = fact.tile([NR, W], BF16)
    nc.vector.scalar_tensor_tensor(yl[:], fyv[:], av[:, 0:1], yh[:],
                                   op0=ALU.mult, op1=ALU.subtract)

    # ---- bounce through DRAM into the 32-aligned 6-row layout ----
    # staging [3, 64, 336]: x first (ready early); y side (yh, yh, yl)
    nc.sync.dma_start(xstg.ap()[0], xh[:])
    nc.scalar.dma_start(xstg.ap()[1], xl[:])
    nc.scalar.dma_start(xstg.ap()[2], xh[:])
    nc.sync.dma_start(ystg.ap()[0], yh[:])
    nc.scalar.dma_start(ystg.ap()[1], yh[:])
    nc.sync.dma_start(ystg.ap()[2], yl[:])

    # gather DMAs: dst rows 32q+3t+u <- stg[u, 8b+2q+t, x]
    FY = [ffac.tile([128, NB, W], BF16, name=f"FY{q}", tag=f"fy{q}")
          for q in range(4)]
    FX = [ffac.tile([128, NB, W], BF16, name=f"FX{q}", tag=f"fx{q}")
          for q in range(4)]
    ya = ystg.ap()  # [3, 64, 336]
    xa = xstg.ap()
    qeng = (nc.sync, nc.scalar, nc.sync, nc.scalar)
    for q in range(4):
        for t in range(2):
            r0 = 32 * q + 3 * t
            qeng[q].dma_start(FY[q][r0:r0 + 3, :, :], ya[:, 2 * q + t::8, :])
            qeng[(q + 1) % 4].dma_start(FX[q][r0:r0 + 3, :, :],
                                        xa[:, 2 * q + t::8, :])

    # output stays in the stage layout [p, m, (c x)] (y = 3p+c): each
    # partition line is one contiguous DRAM run per DMA, so descriptor
    # dispatch (the output-queue bottleneck) halves; host untangles it
    dview = out_t.ap()

    pts = {}
    for j0 in range(0, NMAPS, GROUP):
        mb = small.tile([P, GROUP], F32, tag="mb")
        # coarse matmuls for the whole group first (maximal rg lookahead)
        cts = {}
        for j in range(j0, j0 + GROUP):
            q, b = j % 4, j // 4
            ct = cps.tile([P, 512], F32, tag="ct")
            nc.tensor.matmul(ct[:, 0:CW], FY[q][32 * q:32 * q + 6, b, 0::3],
                             FX[q][32 * q:32 * q + 6, b, 0::2],
                             start=True, stop=True,
                             tile_position=(32 * q, 0))
            cts[j] = ct
        for j in range(j0, j0 + GROUP):
            q, b = j % 4, j // 4
            pt = pmap.tile([P, NCH * 512], F32, tag="pmap")
            rhs = FX[q][32 * q:32 * q + 6, b, :]
            for cix in range(NCH):
                lhsT = FY[q][32 * q:32 * q + 6, b, cix::3]
                nc.tensor.matmul(pt[:, cix * 512:cix * 512 + W], lhsT, rhs,
                                 start=True, stop=True,
                                 tile_position=(32 * q, 0))
            pts[j] = pt
            nc.vector.reduce_max(mb[:, j - j0:j - j0 + 1], cts[j][:, 0:CW],
                                 axis=AX.X)

        # peak chain: rg = 1/(allreduce_max(mb)*PKCORR + EPS)
        par = small.tile([P, GROUP], F32, tag="par")
        nc.gpsimd.partition_all_reduce(par[:], mb[:], channels=P,
                                       reduce_op=bass_isa.ReduceOp.max)
        pke = small.tile([P, GROUP], F32, tag="pke")
        # on ACT (Identity(par*PKCORR + EPS)): keeps the gpsimd wait off
        # the DVE queue, which paces the PSUM-recycle loop
        nc.scalar.activation(pke[:], par[:], AF.Identity, bias=epst[:, 0:1],
                             scale=PKCORR)
        rg = small.tile([P, GROUP], F32, tag="rg")
        nc.vector.reciprocal(rg[:], pke[:])

        # fused scale+cast drain straight out of PSUM: ACT chunks 0-1,
        # DVE chunk 2; then one output DMA per group
        st = sstage.tile([P, GROUP, NCH * W], BF16, tag="sst")
        for j in range(j0, j0 + GROUP):
            gi = j - j0
            pview = pts[j][:].rearrange("p (c z) -> p c z", c=NCH)[:, :, 0:W]
            sview = st[:, gi, :].rearrange("p (c x) -> p c x", c=NCH)
            nc.vector.tensor_scalar_mul(sview[:, 2, :], pview[:, 2, :],
                                        rg[:, gi:gi + 1])
            nc.scalar.mul(sview[:, 0:2, :], pview[:, 0:2, :],
                          rg[:, gi:gi + 1])
            del pts[j]
        nc.sync.dma_start(dview[:, j0:j0 + GROUP, :], st[:])


@functools.lru_cache(maxsize=1)
def _build():
    nc = bacc.Bacc("TRN2", target_bir_lowering=False, debug=False)
    negc_in = nc.dram_tensor("negc", [NR, 2], F32, kind="ExternalInput")
    out_t = nc.dram_tensor("out", [P, NMAPS, NCH * W], BF16,
                           kind="ExternalOutput")

    grid = (np.arange(W, dtype=np.float64) / (W - 1)).astype(np.float32)
    grid_const = nc.inline_tensor(np.tile(grid, (NR, 1)), name="gridc")

    ystg = nc.dram_tensor("ystg", [3, NR, W], BF16)
    xstg = nc.dram_tensor("xstg", [3, NR, W], BF16)

    with tile.TileContext(nc) as tc, ExitStack() as ctx:
        _emit(nc, tc, ctx, negc_in, out_t, grid_const, ystg, xstg)
    nc.compile()
    return nc


def _in_map_for(gaze, hand, b):
    cg = np.asarray(gaze[b], dtype=np.float32).reshape(NMAPS, 2)
    ch = np.asarray(hand[b], dtype=np.float32).reshape(NMAPS, 2)
    inter = np.stack([cg, ch], axis=1).reshape(NR, 2)  # row 2*j + t
    return {"negc": np.ascontiguousarray(-inter)}


def kernel(gaze_coords, hand_coords, _trace=False, **trace_kwargs):
    gaze_coords = np.asarray(gaze_coords, dtype=np.float32)
    hand_coords = np.asarray(hand_coords, dtype=np.float32)
    B = gaze_coords.shape[0]
    assert B == N_CORES, f"expected batch {N_CORES}, got {B}"
    nc = _build()
    in_maps = [_in_map_for(gaze_coords, hand_coords, b) for b in range(B)]
    res = run_bass_kernel_spmd(nc, in_maps, list(range(N_CORES)),
                               trace=_trace, **trace_kwargs)
    def _untangle(o):
        # [P, NMAPS, NCH*W] -> [NMAPS, H, W] with y = 3p+c
        a = np.asarray(o, dtype=np.float32).reshape(P, NMAPS, NCH, W)
        return a.transpose(1, 0, 2, 3).reshape(NMAPS, H, W)

    out = np.stack(
        [_untangle(res.results[i]["out"]).reshape(S_DIM, C_DIM, H, W)
         for i in range(B)],
        axis=0,
    )
    if _trace:
        return out, res
    return out
